# revision 12
# baseline (speedup 1.0000x reference)
"""AdaptiveDilatedConv2d on 8 TRN2 NeuronCores.

Factorization (validated vs reference in numpy):
  out[o,r,s] = bias[o] + sum_t sum_{dy,dx} Wr_t[dy](r,s)*Wc_t[dx](r,s)*Z_t[o,r+dy,s+dx]
  Z_t[o,q] = sum_c weight[o,c,t] * x[c,q]
Stage 1 (PE): Z_t^T[w,o] per image row via matmul (lhsT = x row slice).
Stage 2 (PE): per output row r, 27 banded matmuls over image columns:
  out_row[o,s] = sum_{(t,dy)} sum_q Z_t^T[r+dy][q,o] * B[t,dy,r][q,s]
  where B[q,s] = Wr_t[dy](r,s)*Wc_t[q-s](r,s) for 0<=q-s<=6 else 0
  (host-precomputed bf16 mask tiles, streamed from DRAM).

Sharding: core k handles images (2*(k//2), 2*(k//2)+1), output rows
[63*(k%2), 63*(k%2)+63). Mask tiles are shared by both images on a core.
"""
import numpy as np
import ml_dtypes

import concourse.bass as bass
import concourse.mybir as mybir
import concourse.tile as tile
from concourse import bacc
from concourse.bass import ts
from concourse.bass_utils import run_bass_kernel_spmd

K = 3
C = 128           # in channels
O = 128           # out channels
H = W = 128
Ho = Wo = 126
NIMG = 8
DYMAX = 7         # dy/dx in 0..6
ROWS_HALF = 63    # output rows per core
SLAB = ROWS_HALF + DYMAX  # 70 input rows per core per image
NT = K * K        # 9 taps
WPACK = NT * O    # 1152


def _dkh(k):
    return [0] if k == 0 else ([1, 2, 3] if k == 1 else [2, 3, 4, 5, 6])


# fixed (tap, dy) enumeration shared by host packing and device program.
# Tap (0,0) at dy=0 is identity sampling (mask == I): handled as a direct
# matmul from the x row with W00 stationary, so it is excluded here.
PAIRS = [(kh * K + kw, kh, kw, dy)
         for kh in range(K) for kw in range(K) for dy in _dkh(kh)][1:]
NPAIR = len(PAIRS)  # 26
MCOLS = NPAIR * Wo


def _interp_bilinear(r, out_h, out_w):
    in_h, in_w = r.shape

    def src(n_out, n_in):
        s = (np.arange(n_out, dtype=np.float32) + 0.5) * (n_in / n_out) - 0.5
        return np.clip(s, 0.0, n_in - 1.0)

    sy = src(out_h, in_h)
    sx = src(out_w, in_w)
    y0 = np.floor(sy).astype(np.int32)
    x0 = np.floor(sx).astype(np.int32)
    y1 = np.minimum(y0 + 1, in_h - 1)
    x1 = np.minimum(x0 + 1, in_w - 1)
    wy = (sy - y0)[:, None]
    wx = (sx - x0)[None, :]
    return (r[y0[:, None], x0[None, :]] * (1 - wy) * (1 - wx)
            + r[y0[:, None], x1[None, :]] * (1 - wy) * wx
            + r[y1[:, None], x0[None, :]] * wy * (1 - wx)
            + r[y1[:, None], x1[None, :]] * wy * wx)


def _build_mask_arrays(rates):
    """Wr[k, d, r, s], Wc[k, d, r, s] float32 with OOB zeroing."""
    rate = _interp_bilinear(rates[0, 0].astype(np.float32), Ho, Wo)
    Wr = np.zeros((K, DYMAX, Ho, Wo), np.float32)
    Wc = np.zeros((K, DYMAX, Ho, Wo), np.float32)
    rr = np.arange(Ho)[:, None]
    ss = np.arange(Wo)[None, :]
    for k in range(K):
        u = k * rate
        f = np.floor(u).astype(np.int32)
        w = u - f
        for d in range(DYMAX):
            v = (f == d) * (1 - w) + (f + 1 == d) * w
            Wr[k, d] = v * (rr + d < H)
            Wc[k, d] = v * (ss + d < W)
    return Wr, Wc


def _build_mask_tiles(rates, r0):
    """[ROWS_HALF, 128, MCOLS] bf16 banded mask tiles for rows r0..r0+62."""
    Wr, Wc = _build_mask_arrays(rates)
    out = np.zeros((ROWS_HALF, W, MCOLS), np.float32)
    s = np.arange(Wo)
    for i, (t, kh, kw, dy) in enumerate(PAIRS):
        for dx in range(DYMAX):
            q = s + dx
            valid = q < W
            sv = s[valid]
            # M[r_local, sv] for r = r0 + r_local
            M = (Wr[kh, dy, r0:r0 + ROWS_HALF, :][:, sv]
                 * Wc[kw, dx, r0:r0 + ROWS_HALF, :][:, sv])
            rl = np.arange(ROWS_HALF)[:, None]
            out[rl, q[valid][None, :], (i * Wo + sv)[None, :]] = M
    return out.astype(ml_dtypes.bfloat16)


def build_nc(repeat=1):
    """Build the SPMD program (same for every core)."""
    nc = bacc.Bacc("TRN2", target_bir_lowering=False, debug=False, num_devices=8)
    bf16 = mybir.dt.bfloat16
    f32 = mybir.dt.float32

    x_d = nc.dram_tensor("x", [2, C, SLAB, W], bf16, kind="ExternalInput")
    w_d = nc.dram_tensor("wpack", [C, WPACK], bf16, kind="ExternalInput")
    b_d = nc.dram_tensor("bias", [O, 1], f32, kind="ExternalInput")
    m_d = nc.dram_tensor("masks", [ROWS_HALF, W, MCOLS], bf16, kind="ExternalInput")
    o_d = nc.dram_tensor("out", [2, O, ROWS_HALF, Wo], f32, kind="ExternalOutput")

    with tile.TileContext(nc) as tc:
        with (
            tc.tile_pool(name="xp", bufs=1) as xp,
            tc.tile_pool(name="wp", bufs=1) as wp,
            tc.tile_pool(name="zp", bufs=18) as zp,
            tc.tile_pool(name="mp", bufs=6) as mp,
            tc.tile_pool(name="op", bufs=3) as op,
            tc.tile_pool(name="ps1", bufs=2, space="PSUM") as ps1,
            tc.tile_pool(name="ps2", bufs=2, space="PSUM") as ps2,
        ):
            wt = wp.tile([C, WPACK], bf16, tag="w")
            nc.sync.dma_start(out=wt[:, :], in_=w_d[:, :])
            bt = wp.tile([O, 1], f32, tag="b")
            nc.sync.dma_start(out=bt[:, :], in_=b_d[:, :])

            def body(it):
                xts = []
                xrs = []
                for img in range(2):
                    xt = xp.tile([C, SLAB * W], bf16, tag=f"x{img}")
                    # head chunks land quickly so stage-1 can start early
                    xr = x_d[img].rearrange("c h w -> c (h w)")
                    nc.sync.dma_start(out=xt[:, :2 * W], in_=xr[:, :2 * W])
                    nc.sync.dma_start(out=xt[:, 2 * W:8 * W], in_=xr[:, 2 * W:8 * W])
                    xts.append(xt)
                    xrs.append(xr)

                zrows = [{}, {}]  # img -> h -> tile
                osts = [None, None]
                mts = {}
                # first mask rows stream before the x bulk so the first
                # stage-2 row never waits behind 6MB of input transfers
                for u0 in range(3):
                    mt = mp.tile([W, MCOLS], bf16, tag="m", name=f"mt{u0}")
                    nc.sync.dma_start(out=mt[:, :], in_=m_d[u0])
                    mts[u0] = mt
                for img in range(2):
                    nc.sync.dma_start(out=xts[img][:, 8 * W:24 * W],
                                      in_=xrs[img][:, 8 * W:24 * W])
                    nc.sync.dma_start(out=xts[img][:, 24 * W:],
                                      in_=xrs[img][:, 24 * W:])
                for h in range(SLAB):
                    # prefetch the mask row used at iteration u + 6 a few
                    # iterations early so the first stage-2 never waits
                    u_dma = h - 3
                    if 3 <= u_dma < ROWS_HALF:
                        mt = mp.tile([W, MCOLS], bf16, tag="m", name=f"mt{u_dma}")
                        nc.sync.dma_start(out=mt[:, :], in_=m_d[u_dma])
                        mts[u_dma] = mt
                    for img in range(2):
                        zt = zp.tile([W, WPACK - O], bf16, tag="z")
                        for chunk in range(2):
                            p1 = ps1.tile([W, 512], f32, tag=f"c{chunk}")
                            nc.tensor.matmul(
                                p1[:, :],
                                xts[img][:, ts(h, W)],
                                wt[:, O + chunk * 512:O + (chunk + 1) * 512],
                                start=True, stop=True,
                            )
                            if chunk == 0:
                                nc.vector.tensor_copy(zt[:, ts(chunk, 512)], p1[:, :])
                            else:
                                nc.scalar.copy(zt[:, ts(chunk, 512)], p1[:, :])
                        zrows[img][h] = zt
                    u = h - (DYMAX - 1)
                    if not (0 <= u < ROWS_HALF):
                        continue
                    mt = mts.pop(u)
                    for img in range(2):
                        p2 = ps2.tile([O, Wo], f32, tag="acc")
                        # identity tap (0,0): sample positions are exactly the
                        # pixel grid, so contract the x row directly
                        nc.tensor.matmul(
                            p2[:, :],
                            wt[:, :O],
                            xts[img][:, u * W:u * W + Wo],
                            start=True, stop=False,
                        )
                        for i, (t, kh, kw, dy) in enumerate(PAIRS):
                            nc.tensor.matmul(
                                p2[:, :],
                                zrows[img][u + dy][:, ts(t - 1, O)],
                                mt[:, ts(i, Wo)],
                                start=False, stop=(i == NPAIR - 1),
                            )
                        g = u % 4
                        if g == 0:
                            osts[img] = op.tile([O, 4 * Wo], f32, tag="o",
                                                name=f"ost{img}_{u}")
                        nc.scalar.activation(
                            osts[img][:, ts(g, Wo)], p2[:, :],
                            mybir.ActivationFunctionType.Identity,
                            bias=bt[:, :], scale=1.0,
                        )
                        if g == 3 or u == ROWS_HALF - 1:
                            nc.sync.dma_start(
                                out=o_d[img][:, u - g:u + 1, :],
                                in_=osts[img][:, :(g + 1) * Wo])

            if repeat == 1:
                body(0)
            else:
                with tc.For_i(0, repeat, 1):
                    body(0)

    nc.compile()
    return nc


def _prep_core_inputs(inputs, weight, rates, bias):
    """Returns list of 8 in_maps (host-side shard + mask precompute)."""
    x = np.asarray(inputs)
    wgt = np.asarray(weight)
    b = np.asarray(bias)

    # wpack[c, t*O + o] = weight[o, c, kh, kw],  t = kh*K + kw
    wpack = np.transpose(wgt.reshape(O, C, NT), (1, 2, 0)).reshape(C, NT * O)
    wpack = np.ascontiguousarray(wpack).astype(ml_dtypes.bfloat16)
    b2 = np.ascontiguousarray(b.reshape(O, 1)).astype(np.float32)

    masks_by_half = [_build_mask_tiles(np.asarray(rates), 0),
                     _build_mask_tiles(np.asarray(rates), ROWS_HALF)]

    in_maps = []
    for k in range(8):
        a = 2 * (k // 2)
        half = k % 2
        r0 = ROWS_HALF * half
        slab = np.zeros((2, C, SLAB, W), np.float32)
        avail = min(SLAB, H - r0)
        slab[0, :, :avail, :] = x[a, :, r0:r0 + avail, :]
        slab[1, :, :avail, :] = x[a + 1, :, r0:r0 + avail, :]
        in_maps.append({
            "x": slab.astype(ml_dtypes.bfloat16),
            "wpack": wpack,
            "bias": b2,
            "masks": masks_by_half[half],
        })
    return in_maps


_NC_CACHE = {}


def _get_nc(repeat=1):
    if repeat not in _NC_CACHE:
        _NC_CACHE[repeat] = build_nc(repeat)
    return _NC_CACHE[repeat]


def kernel(inputs, weight, rates, bias):
    nc = _get_nc(1)
    in_maps = _prep_core_inputs(inputs, weight, rates, bias)
    res = run_bass_kernel_spmd(nc, in_maps, core_ids=list(range(8)))
    out = np.zeros((NIMG, O, Ho, Wo), np.float32)
    for k in range(8):
        a = 2 * (k // 2)
        half = k % 2
        r0 = ROWS_HALF * half
        o = res.results[k]["out"]
        out[a, :, r0:r0 + ROWS_HALF, :] = o[0]
        out[a + 1, :, r0:r0 + ROWS_HALF, :] = o[1]
    return out
